# revision 1
# baseline (speedup 1.0000x reference)
"""AdaptiveRankingLoss on 8 Trainium2 NeuronCores (Bass/Tile), upper-triangle v6.

Math
----
reference:  loss = sum_{i<j, |t_i-t_j|>=0.05} 0.5*(w_i+w_j)*relu(-sign(td)*pd + m) / count
            td = t_i - t_j, pd = p_i - p_j, m = ms*0.08*clip(|td|, 0.1, 1.0)

Every per-pair factor is symmetric in i<->j, so each unordered pair is computed
once.  The 64x64 grid of 128-row blocks is covered by a circulant schedule:
row-block I processes column-blocks J in the wrapped window [I, I+n_I) mod 64,
n_I = 33 for I<=31 and 32 for I>=32; every unordered block pair lands in
exactly one window, and the diagonal block leads each window.  Core k owns
blocks {4k..4k+3, 32+4k..32+4k+3}: identical shapes on every core.

Column data is laid out per-core ROTATED by 4k blocks, with the first 3 blocks
duplicated as a tail, so every window is one contiguous slice of a single
[128, 8576] broadcast tile per tensor.

v6 pipeline per block (rows on partitions, window cols on free), bf16.
Engine budget tuned to measured rates: DVE tensor_scalar runs 4x, tensor_tensor
2x, ACT 1x (scalar_tensor_tensor runs 1x on DVE - avoided entirely):
    ACT: ad   = Abs( tq_j - tq_i )        tq = bf16(0.08*ms*t)
    ACT: s    = Sign( tq_i - tq_j )
    DVE: v    = (ad is_ge theta)          theta = 0.004*ms
    DVE: pd   = pq_j - pq_i
    DVE: mg   = ad max lo                 lo = 0.008*ms; upper clip at 0.08*ms
                                          never binds for targets in [0,1)
    DVE: q    = pd * s
    DVE: vp   = q + mg
    ACT: viol[:, :XS]  = Relu(vp)         column-split to balance ACT vs DVE
    DVE: viol[:, XS:]  = vp max 0
    DVE: g    = viol * v
PE does ONE two-column transpose-reduce per 128-col chunk of g:
    ps_col[:, 2c:2c+2] = lhsT=g_chunk @ rhs=[ones, w_row]
giving the plain colsum AND the w_i-weighted colsum in one stationary pass;
sum of weighted colsums = S_r, so no separate row-reduce streams.  The pair
count C is computed host-side by a sorted two-pointer over raw targets.
The diagonal block is computed UNMASKED: its lower triangle equals its upper
triangle exactly (all factors symmetric, bf16 ops commute under sign flip) and
i==j dies via v (ad=0).  Host halves the diag accumulators / diag colsum chunk.

Host combines in f64:
    S_r = sum(Srd)/2 + sum(Srr);  C = sum(Cd)/2 + sum(Cr)
    S_c = sum_slot,c,p colsum[p, 33*slot+c] * w_col[J*128+p] (diag halved)
    loss = 0.5*(S_r + S_c) / C
All t/p/w values are bf16-quantized identically on host for row scalars and
column data so pairwise terms stay exactly symmetric.
"""

import sys

if "/opt/trn_rl_repo" not in sys.path:
    sys.path.insert(0, "/opt/trn_rl_repo")

import numpy as np
import ml_dtypes

N = 8192
P = 128
N_CORES = 8
NBLOCKS_TOTAL = N // P                 # 64 row blocks globally
SLOTS = 8                              # row blocks per core
LC = N + 3 * P                         # 8576 local (rotated) columns
XS = 3520                             # viol column split: [0,XS) ACT, [XS,L) DVE
# per-slot window start / length in the local column layout
SLOT_START = [P * i for i in range(4)] + [N // 2 + P * i for i in range(4)]
SLOT_LEN = [33 * P] * 4 + [32 * P] * 4

_CACHE = {}


def _core_blocks(core):
    return [4 * core + i for i in range(4)] + [32 + 4 * core + i for i in range(4)]


def _window(I):
    n = 33 if I <= 31 else 32
    return [(I + j) % NBLOCKS_TOTAL for j in range(n)]


def _mm_chunks(start, end):
    f = start
    while f < end:
        yield f, min(f + 512, end)
        f = min(f + 512, end)


def _build():
    from contextlib import ExitStack
    from concourse import bacc, tile, mybir

    BF16 = mybir.dt.bfloat16
    F32 = mybir.dt.float32
    Alu = mybir.AluOpType
    Act = mybir.ActivationFunctionType

    nc = bacc.Bacc("TRN2", target_bir_lowering=False, debug=False,
                   num_devices=N_CORES)

    tql_ext = nc.dram_tensor("tql", [P, LC], BF16, kind="ExternalInput").ap()
    pql_ext = nc.dram_tensor("pql", [P, LC], BF16, kind="ExternalInput").ap()
    # aux f32: 0:8 ti | 8:16 nti | 16:24 pi | 32 theta | 33 lo
    aux_ext = nc.dram_tensor("aux", [P, 36], F32, kind="ExternalInput").ap()
    # auxb bf16 [P,16]: col 2b = 1.0, col 2b+1 = row weights of slot b
    auxb_ext = nc.dram_tensor("auxb", [P, 16], BF16, kind="ExternalInput").ap()
    # out f32: [P, 528]: interleaved per chunk [colsum, w-weighted colsum];
    # slots 0-3 in 0:264, slots 4-7 in 264:528
    out_ext = nc.dram_tensor("out", [P, 528], F32, kind="ExternalOutput").ap()

    with tile.TileContext(nc) as tc:
        with ExitStack() as ctx:
            singles = ctx.enter_context(tc.tile_pool(name="singles", bufs=1))
            work = ctx.enter_context(tc.tile_pool(name="work", bufs=2))
            psum = ctx.enter_context(tc.tile_pool(name="psum", bufs=1, space="PSUM"))

            aux_sb = singles.tile([P, 36], F32)
            nc.scalar.dma_start(out=aux_sb[:], in_=aux_ext[:])
            auxb_sb = singles.tile([P, 16], BF16)
            nc.scalar.dma_start(out=auxb_sb[:], in_=auxb_ext[:])

            ones_sb = singles.tile([P, 1], BF16)
            nc.gpsimd.memset(ones_sb[:], 1.0)
            zerob_sb = singles.tile([P, 1], BF16)
            nc.gpsimd.memset(zerob_sb[:], 0.0)

            tqb = singles.tile([P, LC], BF16)
            pqb = singles.tile([P, LC], BF16)
            # column data arrives host-pre-broadcast: plain contiguous DMAs,
            # chunked for early compute start; tqb first (first dependency)
            BCH = LC // 8  # 1072
            for eng, dst, src_ in ((nc.sync, tqb, tql_ext),
                                   (nc.gpsimd, pqb, pql_ext)):
                for c0 in range(0, LC, BCH):
                    sl = slice(c0, c0 + BCH)
                    eng.dma_start(out=dst[:, sl], in_=src_[:, sl])

            ps_cola = psum.tile([P, 264], F32)
            ps_colb = psum.tile([P, 264], F32)
            nc.vector.memset(ps_cola[:], 0.0)
            nc.vector.memset(ps_colb[:], 0.0)

            def _ranges(b):
                # first/last slot split into halves: ramps DVE up earlier at
                # the start, overlaps PE column-reduce with DVE at the end
                L = SLOT_LEN[b]
                if b == 0:
                    qt = (L // 4 // P) * P
                    return [(0, qt), (qt, 2 * qt), (2 * qt, L)]
                if b == SLOTS - 1:
                    h = (L // 2 // P) * P
                    return [(0, h), (h, L)]
                return [(0, L)]

            def emit_adsign(b):
                st, L = SLOT_START[b], SLOT_LEN[b]
                ad = work.tile([P, L], BF16, tag="ad", bufs=3)
                s = work.tile([P, L], BF16, tag="s", bufs=3)
                for c0, c1 in _ranges(b):
                    nc.scalar.activation(out=ad[:, c0:c1],
                                         in_=tqb[:, st + c0:st + c1],
                                         func=Act.Abs,
                                         bias=aux_sb[:, 8 + b:9 + b], scale=1.0)
                    nc.scalar.activation(out=s[:, c0:c1],
                                         in_=tqb[:, st + c0:st + c1],
                                         func=Act.Sign,
                                         bias=aux_sb[:, b:b + 1], scale=-1.0)
                return ad, s

            pend = emit_adsign(0)
            last = {}
            for b in range(SLOTS):
                ad, s = pend
                st, L = SLOT_START[b], SLOT_LEN[b]
                nchunk = L // P
                v = work.tile([P, L], BF16, tag="v", bufs=2)
                pd = work.tile([P, L], BF16, tag="pd", bufs=2)
                mg = work.tile([P, L], BF16, tag="mg", bufs=2)
                q = work.tile([P, L], BF16, tag="q", bufs=2)
                vp = work.tile([P, L], BF16, tag="vp", bufs=2)
                viol = work.tile([P, L], BF16, tag="viol", bufs=2)
                g = work.tile([P, L], BF16, tag="g", bufs=2)
                for ri, (c0, c1) in enumerate(_ranges(b)):
                    r = slice(c0, c1)
                    nc.vector.tensor_scalar(
                        out=v[:, r], in0=ad[:, r], scalar1=aux_sb[:, 32:33],
                        scalar2=None, op0=Alu.is_ge)
                    nc.vector.tensor_scalar(
                        out=pd[:, r], in0=pqb[:, st + c0:st + c1],
                        scalar1=aux_sb[:, 16 + b:17 + b],
                        scalar2=None, op0=Alu.subtract)
                    nc.vector.tensor_scalar(
                        out=mg[:, r], in0=ad[:, r], scalar1=aux_sb[:, 33:34],
                        scalar2=None, op0=Alu.max)
                    nc.vector.tensor_tensor(out=q[:, r], in0=pd[:, r],
                                            in1=s[:, r], op=Alu.mult)
                    nc.vector.tensor_tensor(out=vp[:, r], in0=q[:, r],
                                            in1=mg[:, r], op=Alu.add)
                    if b + 1 < SLOTS and ri == len(_ranges(b)) - 1:
                        pend = emit_adsign(b + 1)
                    # relu, column-split across ACT and DVE for engine balance
                    a0, a1 = c0, min(c1, XS)
                    if a1 > a0:
                        nc.scalar.activation(out=viol[:, a0:a1],
                                             in_=vp[:, a0:a1], func=Act.Relu)
                    d0, d1 = max(c0, XS), c1
                    if d1 > d0:
                        nc.vector.tensor_scalar(
                            out=viol[:, d0:d1], in0=vp[:, d0:d1], scalar1=0.0,
                            scalar2=None, op0=Alu.max)
                    nc.vector.tensor_tensor(out=g[:, r], in0=viol[:, r],
                                            in1=v[:, r], op=Alu.mult)

                # per-chunk transpose-reduce with TWO rhs columns:
                # [colsum, w_i-weighted colsum] in one stationary pass.
                # S_r = sum of weighted colsums, so no row-reduce streams.
                ps_c = ps_cola if b < 4 else ps_colb
                o0 = 66 * (b % 4)
                for c in range(nchunk):
                    nc.tensor.matmul(
                        ps_c[:, o0 + 2 * c:o0 + 2 * c + 2],
                        lhsT=g[:, c * P:(c + 1) * P],
                        rhs=auxb_sb[:, 2 * b:2 * b + 2],
                        start=True, stop=True)
                last = {"g": g, "v": v}

            # drain pushers: push PE->PSUM writeback of the last real writes
            # through before the reads below (unused columns of slots w/ 32
            # chunks: local cols 64:66 of each 66-block)
            pushers = []
            for uc in (64, 130, 196, 262):
                # slots 4-7 use only 64 of their 66 ps_colb column slots;
                # ps_cola (33-chunk slots) has NO unused columns
                pushers.append(nc.tensor.matmul(
                    ps_colb[:, uc:uc + 2], lhsT=last["g"][:, 0:P],
                    rhs=auxb_sb[:, 0:2], start=True, stop=True))

            out_sb = singles.tile([P, 528], F32)
            c0 = nc.scalar.copy(out=out_sb[:, 0:264], in_=ps_cola[:])
            c1 = nc.scalar.copy(out=out_sb[:, 264:528], in_=ps_colb[:])
            for cc in (c0, c1):
                for pp in pushers:
                    tile.add_dep_helper(cc.ins, pp.ins,
                                        reason="final copy waits drain pushers")
            nc.sync.dma_start(out=out_ext[:], in_=out_sb[:])

    nc.compile()
    return nc


def _get_nc():
    if "nc" not in _CACHE:
        _CACHE["nc"] = _build()
    return _CACHE["nc"]


def _prepare_in_maps(predictions, targets, snr_weights, margin_scale):
    ms = float(margin_scale)
    bf16 = ml_dtypes.bfloat16

    t = np.asarray(targets, np.float32)
    p = np.asarray(predictions, np.float32)
    w = np.asarray(snr_weights, np.float32)

    # bf16-quantize once; identical values feed column data and row scalars so
    # every pairwise term is exactly symmetric.
    tq = (0.08 * ms * t).astype(bf16)
    pq = p.astype(bf16)
    wq = w.astype(bf16)
    tqf = tq.astype(np.float32)
    pqf = pq.astype(np.float32)

    in_maps = []
    for core in range(N_CORES):
        rot = 4 * core * P
        # rotated layout + 3-block tail so every window is contiguous
        idx = (rot + np.arange(LC)) % N
        tql = np.ascontiguousarray(
            np.broadcast_to(tq[idx].reshape(1, LC), (P, LC)))
        pql = np.ascontiguousarray(
            np.broadcast_to(pq[idx].reshape(1, LC), (P, LC)))
        blocks = _core_blocks(core)
        ti = np.empty((P, SLOTS), np.float32)
        pi = np.empty((P, SLOTS), np.float32)
        wib = np.empty((P, SLOTS), np.float32)
        for slot, I in enumerate(blocks):
            rows = slice(I * P, (I + 1) * P)
            ti[:, slot] = tqf[rows]
            pi[:, slot] = pqf[rows]
            wib[:, slot] = wq[rows]
        cst = np.zeros((P, 4), np.float32)
        cst[:, 0] = np.float32(0.05 * 0.08 * ms)   # theta
        cst[:, 1] = np.float32(0.1 * 0.08 * ms)    # lo
        aux = np.concatenate([ti, -ti, pi, -pi, cst], axis=1)
        auxw = np.empty((P, 2 * SLOTS), np.float32)
        auxw[:, 0::2] = 1.0
        auxw[:, 1::2] = wib
        in_maps.append({"tql": tql, "pql": pql,
                        "aux": aux.astype(np.float32),
                        "auxb": auxw.astype(bf16)})
    return in_maps


def _numpy_fallback(predictions, targets, snr_weights, margin_scale):
    t = np.asarray(targets, np.float64)
    p = np.asarray(predictions, np.float64)
    w = np.asarray(snr_weights, np.float64)
    ms = float(margin_scale)
    total = 0.0
    count = 0
    for i0 in range(0, N, 512):
        i1 = min(i0 + 512, N)
        td = t[i0:i1, None] - t[None, :]
        ad = np.abs(td)
        upper = (np.arange(i0, i1)[:, None] < np.arange(N)[None, :])
        valid = upper & (ad >= 0.05)
        margin = ms * 0.08 * np.clip(ad, 0.1, 1.0)
        pdm = p[i0:i1, None] - p[None, :]
        viol = np.maximum(-np.sign(td) * pdm + margin, 0.0)
        pw = 0.5 * (w[i0:i1, None] + w[None, :])
        total += float((pw * viol)[valid].sum())
        count += int(valid.sum())
    return np.float32(total / count if count > 0 else 0.0)


def kernel(predictions, targets, snr_weights, margin_scale):
    from concourse.bass_utils import run_bass_kernel_spmd

    if float(margin_scale) <= 0.0:
        return _numpy_fallback(predictions, targets, snr_weights, margin_scale)

    nc = _get_nc()
    in_maps = _prepare_in_maps(predictions, targets, snr_weights, margin_scale)
    res = run_bass_kernel_spmd(nc, in_maps, core_ids=list(range(N_CORES)))

    bf16 = ml_dtypes.bfloat16
    wqf = np.asarray(snr_weights, np.float32).astype(bf16).astype(np.float64)

    # count on host: pairs with t_i - t_j >= 0.05 via sorted two-pointer
    # (reference f32/f64 semantics; boundary mismatch vs the device's bf16
    # mask is ~1e-5 of pairs)
    ts_sorted = np.sort(np.asarray(targets, np.float64))
    C = float(np.searchsorted(ts_sorted, ts_sorted - 0.05, side="right").sum())

    S_r = 0.0
    S_c = 0.0
    for core in range(N_CORES):
        o = np.asarray(res.results[core]["out"], np.float64)
        blocks = _core_blocks(core)
        for slot, I in enumerate(blocks):
            base = 264 * (slot // 4) + 66 * (slot % 4)
            win = _window(I)
            for c, J in enumerate(win):
                w_col = wqf[J * P:(J + 1) * P]
                scale = 0.5 if c == 0 else 1.0
                S_c += scale * float(w_col @ o[:, base + 2 * c])
                S_r += scale * float(o[:, base + 2 * c + 1].sum())
    loss = 0.5 * (S_r + S_c) / C if C > 0 else 0.0
    return np.float32(loss)



# revision 13
# speedup vs baseline: 2.7483x; 2.7483x over previous
"""AdaptiveRankingLoss on 8 Trainium2 NeuronCores (Bass/Tile), v7 "sorted-u".

Math
----
reference:  loss = sum_{i<j, |t_i-t_j|>=0.05} 0.5*(w_i+w_j)*relu(-sign(td)*pd + m) / count
            td = t_i - t_j, pd = p_i - p_j, m = ms*0.08*clip(|td|, 0.1, 1.0)

v7 key idea: sort everything by target on the host.  In sorted order, for a
pair (i, j) with sorted ranks r_i < r_j, the violation is
    viol = relu(p_i - p_j + m),   m = max(ad, lo),  ad = tq_j - tq_i >= 0
with tq = 0.08*ms*t.  Define u = p - tq.  Whenever the pair is guaranteed
valid (ad >= theta) and unclipped (ad >= lo), m == ad and
    viol = relu(u_i - u_j)          -- ONE fused element op.
Using relu(s*x) = (s*x + |x|)/2, the device only computes h = |u_i - u_j|
(orientation-free!) and the host adds the exact bilinear term
sum 0.5*(w_i+w_j)*s*(u_i-u_j) from per-block prefix sums in f64.

The 64x64 grid of 128-row blocks uses the v6 circulant schedule: row-block I
processes column-blocks J = (I+d) % 64 for d in [0, n_I), n_I = 33 (I<=31)
else 32; each unordered block pair lands in exactly one window.  Core k owns
blocks {4k..4k+3, 32+4k..32+4k+3}; column data is laid out per-core rotated
by 4k blocks with a 3-block duplicated tail, so every window is contiguous.

Per-window distance bands (host-verified predicates, shared program):
    d in [0, D0):  skip    -- every pair invalid (|dt| < 0.05)
    d in [D0,D1):  full    -- masked chain (some pairs invalid)
    d in [D1,D2):  mid     -- all valid, margin may clip (no mask)
    d in [D2, n):  far     -- all valid, unclipped: h = |u_i - u_j| only
    wrapped cols (I+d >= 64) at d < D2: covered by an extra "nearx" abs
    range on slots 4-7; the near-chain mask v = (tq_j >= t_i + theta) is
    one-sided so wrapped cols contribute exactly 0 there.  Chunks whose
    band treatment does not match their true class are simply ignored by
    the host (each block pair is USED from exactly one chunk).

Near chain (bf16, rows on partitions):
    DVE ts : m    = abs_max(tq_j - ti, lo)        [d in [D0,D2)]
    DVE ts : pdn  = pq_j - pi
    DVE tt : vp   = m - pdn
    DVE ts : viol = max(vp, 0)
    DVE ts : v    = (tq_j is_ge ti+theta)         [d in [D0,D1) only]
    DVE tt : g    = viol * v                      [d in [D0,D1) only]
Far/nearx: h = |uq_j - u_i| as DVE ts-dual (subtract, abs_max 0) on a
~17% column split and ACT Abs(bias=-u_i) on the rest (engine balance).

PE reduces every 128-col chunk with rhs=[ones, w_row]: ps[:,2c:2c+2] =
[colsum, w_i-weighted colsum].  Host combines in f64:
    near chunk:  0.5*(w_col @ colsum + sum(wcolsum))
    far  chunk:  0.5*( sigma*B(I,J) + 0.5*(w_col @ colsum + sum(wcolsum)) )
with B(I,J) = 0.5*(128*Swu_I + Sw_J*Su_I - Sw_I*Su_J - 128*Swu_J) from
per-block sums of the SAME bf16-quantized u and w, sigma = -1 iff wrapped.
Count C is exact on host (sorted two-pointer over raw targets).
"""

import sys

if "/opt/trn_rl_repo" not in sys.path:
    sys.path.insert(0, "/opt/trn_rl_repo")

import numpy as np
import ml_dtypes

N = 8192
P = 128
N_CORES = 8
NBLOCKS_TOTAL = N // P                 # 64 row blocks globally
SLOTS = 8                              # row blocks per core
LC = N + 3 * P                         # 8576 local (rotated) columns
SLOT_START = [P * i for i in range(4)] + [N // 2 + P * i for i in range(4)]
SLOT_NWIN = [33] * 4 + [32] * 4
D0, D1, D2 = 2, 5, 8                   # band thresholds (verified on input)
# far chunks put on DVE (rest on ACT) per slot, for DVE/ACT balance
FAR_DVE = [5] * 4 + [4] * 4

_CACHE = {}


def _core_block(core, slot):
    return 4 * core + slot if slot < 4 else 32 + 4 * core + (slot - 4)


def _nearx_d0(slot):
    # wrapped cols with d < D2 exist only for I >= 64 - D2 + 1, i.e. slots
    # 4..7 on the highest cores; conservative shared range [4-(slot-4), D2)
    return 4 - (slot - 4)


def _chunk_table():
    """Per-slot chunk list in PSUM emission order: (kind, d).

    kind: 'N' near band d in [D0, D2); 'F' far; 'X' nearx (slots 4-7).
    Returns (table, bases, total_cols): table[slot] = list[(kind, d)],
    bases[slot] = psum column base (2 cols per chunk), A tile = slots 0-3
    cols [0, baseA_end), B tile = slots 4-7.
    """
    table = []
    for b in range(SLOTS):
        lst = [("N", d) for d in range(D0, D2)]
        lst += [("F", d) for d in range(D2, SLOT_NWIN[b])]
        if b >= 4:
            lst += [("X", d) for d in range(_nearx_d0(b), D2)]
        table.append(lst)
    basesA = []
    off = 0
    for b in range(4):
        basesA.append(off)
        off += 2 * len(table[b])
    a_end = off
    basesB = []
    off = 0
    for b in range(4, 8):
        basesB.append(off)
        off += 2 * len(table[b])
    b_end = off
    return table, basesA + basesB, a_end, b_end


def _build(ms):
    from contextlib import ExitStack
    from concourse import bacc, tile, mybir

    BF16 = mybir.dt.bfloat16
    F32 = mybir.dt.float32
    Alu = mybir.AluOpType
    Act = mybir.ActivationFunctionType

    table, bases, a_end, b_end = _chunk_table()

    nc = bacc.Bacc("TRN2", target_bir_lowering=False, debug=False,
                   num_devices=N_CORES)

    tql_ext = nc.dram_tensor("tql", [P, LC], BF16, kind="ExternalInput").ap()
    pql_ext = nc.dram_tensor("pql", [P, LC], BF16, kind="ExternalInput").ap()
    uql_ext = nc.dram_tensor("uql", [P, LC], BF16, kind="ExternalInput").ap()
    # aux f32 [P, 40]: 0:8 ti | 8:16 pi | 16:24 ui | 24:32 -ui | 32:40 ti+theta
    aux_ext = nc.dram_tensor("aux", [P, 40], F32, kind="ExternalInput").ap()
    # auxb bf16 [P,16]: col 2b = 1.0, col 2b+1 = row weights of slot b
    auxb_ext = nc.dram_tensor("auxb", [P, 16], BF16, kind="ExternalInput").ap()
    OUTC = a_end + b_end
    out_ext = nc.dram_tensor("out", [P, OUTC], F32, kind="ExternalOutput").ap()

    # lo is a compile-time const in tq-units (ms folded into tq on host)
    lo_c = float(0.1 * 0.08 * ms)

    if True:
        with tile.TileContext(nc) as tc:
            with ExitStack() as ctx:
                singles = ctx.enter_context(tc.tile_pool(name="singles", bufs=1))
                work = ctx.enter_context(tc.tile_pool(name="work", bufs=2))
                psum = ctx.enter_context(tc.tile_pool(name="psum", bufs=1,
                                                      space="PSUM"))

                aux_sb = singles.tile([P, 40], F32)
                nc.sync.dma_start(out=aux_sb[:], in_=aux_ext[:])
                auxb_sb = singles.tile([P, 16], BF16)
                nc.sync.dma_start(out=auxb_sb[:], in_=auxb_ext[:])

                tqb = singles.tile([P, LC], BF16)
                pqb = singles.tile([P, LC], BF16)
                uqb = singles.tile([P, LC], BF16)

                # near-band data first (tq, pq for d in [D0, D2) per slot),
                # then uq ranges in slot processing order.
                near_ranges = [(P * (0 + D0), P * (3 + D2)),
                               (N // 2 + P * (0 + D0), N // 2 + P * (3 + D2))]
                for r0, r1 in near_ranges:
                    h = (r0 + r1) // 2
                    nc.sync.dma_start(out=tqb[:, r0:h], in_=tql_ext[:, r0:h])
                    nc.gpsimd.dma_start(out=tqb[:, h:r1], in_=tql_ext[:, h:r1])
                for r0, r1 in near_ranges:
                    h = (r0 + r1) // 2
                    nc.sync.dma_start(out=pqb[:, r0:h], in_=pql_ext[:, r0:h])
                    nc.gpsimd.dma_start(out=pqb[:, h:r1], in_=pql_ext[:, h:r1])
                # uq union [1024, 8576) in 8 chunks, alternating queues
                U0, U1 = P * 8, LC
                step = (U1 - U0) // 8
                for ci in range(8):
                    c0 = U0 + ci * step
                    c1 = U1 if ci == 7 else c0 + step
                    eng = nc.sync if ci % 2 == 0 else nc.gpsimd
                    eng.dma_start(out=uqb[:, c0:c1], in_=uql_ext[:, c0:c1])

                ps_a = psum.tile([P, a_end + 2], F32)
                ps_b = psum.tile([P, b_end + 2], F32)

                for b in range(SLOTS):
                    st = SLOT_START[b]
                    nwin = SLOT_NWIN[b]
                    ps = ps_a if b < 4 else ps_b
                    base = bases[b]

                    NEARC = (D2 - D0) * P       # 768 near cols
                    FULLC = (D1 - D0) * P       # 384 masked cols
                    n0 = st + D0 * P

                    m_t = work.tile([P, NEARC], BF16, tag="m")
                    pdn = work.tile([P, NEARC], BF16, tag="pdn")
                    vp = work.tile([P, NEARC], BF16, tag="vp")
                    viol = work.tile([P, NEARC], BF16, tag="viol")
                    v_t = work.tile([P, FULLC], BF16, tag="v")
                    g_t = work.tile([P, FULLC], BF16, tag="g")

                    # sorted order: tq_j >= ti on non-wrapped near cols
                    nc.vector.tensor_scalar(
                        out=m_t[:], in0=tqb[:, n0:n0 + NEARC],
                        scalar1=aux_sb[:, b:b + 1], scalar2=lo_c,
                        op0=Alu.subtract, op1=Alu.max)
                    nc.vector.tensor_scalar(
                        out=pdn[:], in0=pqb[:, n0:n0 + NEARC],
                        scalar1=aux_sb[:, 8 + b:9 + b], scalar2=None,
                        op0=Alu.subtract)
                    nc.vector.tensor_tensor(
                        out=vp[:], in0=m_t[:], in1=pdn[:], op=Alu.subtract)
                    nc.vector.tensor_scalar(
                        out=viol[:], in0=vp[:], scalar1=0.0, scalar2=None,
                        op0=Alu.max)
                    nc.vector.tensor_scalar(
                        out=v_t[:], in0=tqb[:, n0:n0 + FULLC],
                        scalar1=aux_sb[:, 32 + b:33 + b], scalar2=None,
                        op0=Alu.is_ge)
                    nc.vector.tensor_tensor(
                        out=g_t[:], in0=viol[:, 0:FULLC], in1=v_t[:],
                        op=Alu.mult)

                    # far: h = |uq_j - u_i|, split ACT (bulk) / DVE (tail)
                    nfar = nwin - D2
                    FARC = nfar * P
                    f0 = st + D2 * P
                    h_t = work.tile([P, FARC], BF16, tag="h")
                    nd = FAR_DVE[b]
                    splitc = (nfar - nd) * P
                    # ACT: relu(u_i - uq_j) (true viol on non-wrapped cols)
                    nc.scalar.activation(
                        out=h_t[:, 0:splitc], in_=uqb[:, f0:f0 + splitc],
                        func=Act.Relu,
                        bias=aux_sb[:, 16 + b:17 + b], scale=-1.0)
                    # DVE: relu(uq_j - u_i) (true viol on wrapped cols)
                    nc.vector.tensor_scalar(
                        out=h_t[:, splitc:FARC],
                        in0=uqb[:, f0 + splitc:f0 + FARC],
                        scalar1=aux_sb[:, 16 + b:17 + b], scalar2=0.0,
                        op0=Alu.subtract, op1=Alu.max)

                    hx_t = None
                    if b >= 4:
                        xd0 = _nearx_d0(b)
                        XC = (D2 - xd0) * P
                        x0 = st + xd0 * P
                        hx_t = work.tile([P, XC], BF16, tag="hx")
                        nc.vector.tensor_scalar(
                            out=hx_t[:], in0=uqb[:, x0:x0 + XC],
                            scalar1=aux_sb[:, 16 + b:17 + b], scalar2=0.0,
                            op0=Alu.subtract, op1=Alu.max)

                    # PE reduce per chunk, in _chunk_table order
                    for c, (kind, d) in enumerate(table[b]):
                        if kind == "N":
                            ci = d - D0
                            src = (g_t[:, ci * P:(ci + 1) * P] if d < D1 else
                                   viol[:, ci * P:(ci + 1) * P])
                        elif kind == "F":
                            ci = d - D2
                            src = h_t[:, ci * P:(ci + 1) * P]
                        else:
                            ci = d - _nearx_d0(b)
                            src = hx_t[:, ci * P:(ci + 1) * P]
                        nc.tensor.matmul(
                            ps[:, base + 2 * c:base + 2 * c + 2],
                            lhsT=src, rhs=auxb_sb[:, 2 * b:2 * b + 2],
                            start=True, stop=True)

                # drain pushers into the 2 spare cols of each psum tile
                pushers = []
                for ps, spare, b in ((ps_a, a_end, 3), (ps_b, b_end, 7)):
                    pushers.append(nc.tensor.matmul(
                        ps[:, spare:spare + 2],
                        lhsT=uqb[:, U0:U0 + P],
                        rhs=auxb_sb[:, 2 * b:2 * b + 2],
                        start=True, stop=True))

                out_sb = singles.tile([P, OUTC], F32)
                c0 = nc.scalar.copy(out=out_sb[:, 0:a_end], in_=ps_a[:, 0:a_end])
                c1 = nc.scalar.copy(out=out_sb[:, a_end:OUTC],
                                    in_=ps_b[:, 0:b_end])
                for cc in (c0, c1):
                    for pp in pushers:
                        tile.add_dep_helper(cc.ins, pp.ins,
                                            reason="final copy waits pushers")
                nc.sync.dma_start(out=out_ext[:], in_=out_sb[:])

    nc.compile()
    return nc


def _get_nc(ms=1.0):
    key = ("nc", float(ms))
    if key not in _CACHE:
        _CACHE[key] = _build(float(ms))
    return _CACHE[key]


def _sorted_quantized(predictions, targets, snr_weights, margin_scale):
    ms = float(margin_scale)
    bf16 = ml_dtypes.bfloat16
    t = np.asarray(targets, np.float32)
    p = np.asarray(predictions, np.float32)
    w = np.asarray(snr_weights, np.float32)
    order = np.argsort(t, kind="stable")
    tso, pso, wso = t[order], p[order], w[order]
    tq = (0.08 * ms * tso).astype(bf16)
    tqf = tq.astype(np.float32)
    uq = (pso - tqf).astype(bf16)
    pq = pso.astype(bf16)
    wq = wso.astype(bf16)
    return tso, tq, uq, pq, wq, ms


def _check_bands(tso):
    """Verify the compiled band predicates on the actual sorted targets."""
    ts = tso.astype(np.float64)
    NB = NBLOCKS_TOTAL
    for I in range(NB):
        nwin = 33 if I <= 31 else 32
        for d in range(nwin):
            J = I + d
            if J >= NB:
                # wrapped cols must be far-class; nearx coverage of wrapped
                # d < D2 is structural (d >= 64 - I >= _nearx_d0(slot))
                if ts[I * P] - ts[(J - NB) * P + P - 1] < 0.1:
                    return False
                continue
            if d < D0:
                if ts[J * P + P - 1] - ts[I * P] >= 0.05:
                    return False
            elif D1 <= d < D2:
                if ts[J * P] - ts[I * P + P - 1] < 0.05:
                    return False
            elif d >= D2:
                if ts[J * P] - ts[I * P + P - 1] < 0.1:
                    return False
    return True


def _prepare_in_maps(predictions, targets, snr_weights, margin_scale):
    bf16 = ml_dtypes.bfloat16
    tso, tq, uq, pq, wq, ms = _sorted_quantized(
        predictions, targets, snr_weights, margin_scale)
    tqf = tq.astype(np.float32)
    uqf = uq.astype(np.float32)
    pqf = pq.astype(np.float32)
    theta = np.float32(0.05 * 0.08 * ms)

    in_maps = []
    for core in range(N_CORES):
        rot = 4 * core * P
        idx = (rot + np.arange(LC)) % N
        tql = np.ascontiguousarray(
            np.broadcast_to(tq[idx].reshape(1, LC), (P, LC)))
        pql = np.ascontiguousarray(
            np.broadcast_to(pq[idx].reshape(1, LC), (P, LC)))
        uql = np.ascontiguousarray(
            np.broadcast_to(uq[idx].reshape(1, LC), (P, LC)))
        ti = np.empty((P, SLOTS), np.float32)
        pi = np.empty((P, SLOTS), np.float32)
        ui = np.empty((P, SLOTS), np.float32)
        wib = np.empty((P, SLOTS), np.float32)
        for slot in range(SLOTS):
            I = _core_block(core, slot)
            rows = slice(I * P, (I + 1) * P)
            ti[:, slot] = tqf[rows]
            pi[:, slot] = pqf[rows]
            ui[:, slot] = uqf[rows]
            wib[:, slot] = wq[rows].astype(np.float32)
        aux = np.concatenate([ti, pi, ui, -ui, ti + theta], axis=1)
        auxw = np.empty((P, 2 * SLOTS), np.float32)
        auxw[:, 0::2] = 1.0
        auxw[:, 1::2] = wib
        in_maps.append({"tql": tql, "pql": pql, "uql": uql,
                        "aux": aux.astype(np.float32),
                        "auxb": auxw.astype(bf16)})
    return in_maps


def _numpy_fallback(predictions, targets, snr_weights, margin_scale):
    t = np.asarray(targets, np.float64)
    p = np.asarray(predictions, np.float64)
    w = np.asarray(snr_weights, np.float64)
    ms = float(margin_scale)
    total = 0.0
    count = 0
    for i0 in range(0, N, 512):
        i1 = min(i0 + 512, N)
        td = t[i0:i1, None] - t[None, :]
        ad = np.abs(td)
        upper = (np.arange(i0, i1)[:, None] < np.arange(N)[None, :])
        valid = upper & (ad >= 0.05)
        margin = ms * 0.08 * np.clip(ad, 0.1, 1.0)
        pdm = p[i0:i1, None] - p[None, :]
        viol = np.maximum(-np.sign(td) * pdm + margin, 0.0)
        pw = 0.5 * (w[i0:i1, None] + w[None, :])
        total += float((pw * viol)[valid].sum())
        count += int(valid.sum())
    return np.float32(total / count if count > 0 else 0.0)


def kernel(predictions, targets, snr_weights, margin_scale):
    from concourse.bass_utils import run_bass_kernel_spmd

    if float(margin_scale) <= 0.0:
        return _numpy_fallback(predictions, targets, snr_weights, margin_scale)

    tso, tq, uq, pq, wq, ms = _sorted_quantized(
        predictions, targets, snr_weights, margin_scale)
    if not _check_bands(tso):
        return _numpy_fallback(predictions, targets, snr_weights, margin_scale)

    nc = _get_nc(ms)
    in_maps = _prepare_in_maps(predictions, targets, snr_weights, margin_scale)
    res = run_bass_kernel_spmd(nc, in_maps, core_ids=list(range(N_CORES)))

    uqd = uq.astype(np.float64)
    wqd = wq.astype(np.float64)
    Su = uqd.reshape(NBLOCKS_TOTAL, P).sum(axis=1)
    Sw = wqd.reshape(NBLOCKS_TOTAL, P).sum(axis=1)
    Swu = (wqd * uqd).reshape(NBLOCKS_TOTAL, P).sum(axis=1)

    table, bases, a_end, b_end = _chunk_table()

    # exact pair count via sorted two-pointer over raw targets (f64)
    ts_sorted = np.sort(np.asarray(targets, np.float64))
    C = float(np.searchsorted(ts_sorted, ts_sorted - 0.05, side="right").sum())

    total = 0.0
    for core in range(N_CORES):
        o = np.asarray(res.results[core]["out"], np.float64)
        for b in range(SLOTS):
            I = _core_block(core, b)
            off = bases[b] + (0 if b < 4 else a_end)
            for c, (kind, d) in enumerate(table[b]):
                wrapped = (I + d) >= NBLOCKS_TOTAL
                if kind == "N" and wrapped:
                    continue
                if kind == "X" and not wrapped:
                    continue
                J = (I + d) % NBLOCKS_TOTAL
                colsum = o[:, off + 2 * c]
                wcolsum = o[:, off + 2 * c + 1]
                wcol = wqd[J * P:(J + 1) * P]
                A = 0.5 * (wcol @ colsum + wcolsum.sum())
                if kind == "N":
                    total += A
                else:
                    # device computed relu(sigma_dev*x): ACT chunks +x,
                    # DVE/nearx chunks -x.  Correct mismatched orientation
                    # with the exact bilinear B = sum w_bar * x.
                    if kind == "F":
                        nfar = SLOT_NWIN[b] - D2
                        on_act = (d - D2) < (nfar - FAR_DVE[b])
                    else:
                        on_act = False
                    B = 0.5 * (P * Swu[I] + Sw[J] * Su[I]
                               - Sw[I] * Su[J] - P * Swu[J])
                    if on_act:
                        corr = -B if wrapped else 0.0
                    else:
                        corr = B if not wrapped else 0.0
                    total += A + corr

    loss = total / C if C > 0 else 0.0
    return np.float32(loss)


# revision 18
# speedup vs baseline: 2.9665x; 1.0794x over previous
"""AdaptiveRankingLoss on 8 Trainium2 NeuronCores (Bass/Tile), v7 "sorted-u".

Math
----
reference:  loss = sum_{i<j, |t_i-t_j|>=0.05} 0.5*(w_i+w_j)*relu(-sign(td)*pd + m) / count
            td = t_i - t_j, pd = p_i - p_j, m = ms*0.08*clip(|td|, 0.1, 1.0)

v7 key idea: sort everything by target on the host.  In sorted order, for a
pair (i, j) with sorted ranks r_i < r_j, the violation is
    viol = relu(p_i - p_j + m),   m = max(ad, lo),  ad = tq_j - tq_i >= 0
with tq = 0.08*ms*t.  Define u = p - tq.  Whenever the pair is guaranteed
valid (ad >= theta) and unclipped (ad >= lo), m == ad and
    viol = relu(u_i - u_j)          -- ONE fused element op.
Using relu(s*x) = (s*x + |x|)/2, the device only computes h = |u_i - u_j|
(orientation-free!) and the host adds the exact bilinear term
sum 0.5*(w_i+w_j)*s*(u_i-u_j) from per-block prefix sums in f64.

The 64x64 grid of 128-row blocks uses the v6 circulant schedule: row-block I
processes column-blocks J = (I+d) % 64 for d in [0, n_I), n_I = 33 (I<=31)
else 32; each unordered block pair lands in exactly one window.  Core k owns
blocks {4k..4k+3, 32+4k..32+4k+3}; column data is laid out per-core rotated
by 4k blocks with a 3-block duplicated tail, so every window is contiguous.

Per-window distance bands (host-verified predicates, shared program):
    d in [0, D0):  skip    -- every pair invalid (|dt| < 0.05)
    d in [D0,D1):  full    -- masked chain (some pairs invalid)
    d in [D1,D2):  mid     -- all valid, margin may clip (no mask)
    d in [D2, n):  far     -- all valid, unclipped: h = |u_i - u_j| only
    wrapped cols (I+d >= 64) at d < D2: covered by an extra "nearx" abs
    range on slots 4-7; the near-chain mask v = (tq_j >= t_i + theta) is
    one-sided so wrapped cols contribute exactly 0 there.  Chunks whose
    band treatment does not match their true class are simply ignored by
    the host (each block pair is USED from exactly one chunk).

Near chain (bf16, rows on partitions):
    DVE ts : m    = abs_max(tq_j - ti, lo)        [d in [D0,D2)]
    DVE ts : pdn  = pq_j - pi
    DVE tt : vp   = m - pdn
    DVE ts : viol = max(vp, 0)
    DVE ts : v    = (tq_j is_ge ti+theta)         [d in [D0,D1) only]
    DVE tt : g    = viol * v                      [d in [D0,D1) only]
Far/nearx: h = |uq_j - u_i| as DVE ts-dual (subtract, abs_max 0) on a
~17% column split and ACT Abs(bias=-u_i) on the rest (engine balance).

PE reduces every 128-col chunk with rhs=[ones, w_row]: ps[:,2c:2c+2] =
[colsum, w_i-weighted colsum].  Host combines in f64:
    near chunk:  0.5*(w_col @ colsum + sum(wcolsum))
    far  chunk:  0.5*( sigma*B(I,J) + 0.5*(w_col @ colsum + sum(wcolsum)) )
with B(I,J) = 0.5*(128*Swu_I + Sw_J*Su_I - Sw_I*Su_J - 128*Swu_J) from
per-block sums of the SAME bf16-quantized u and w, sigma = -1 iff wrapped.
Count C is exact on host (sorted two-pointer over raw targets).
"""

import sys

if "/opt/trn_rl_repo" not in sys.path:
    sys.path.insert(0, "/opt/trn_rl_repo")

import numpy as np
import ml_dtypes

N = 8192
P = 128
N_CORES = 8
NBLOCKS_TOTAL = N // P                 # 64 row blocks globally
SLOTS = 8                              # row blocks per core
LC = N + 3 * P                         # 8576 local (rotated) columns
SLOT_START = [P * i for i in range(4)] + [N // 2 + P * i for i in range(4)]
SLOT_NWIN = [33] * 4 + [32] * 4
D0, D1, D2 = 2, 5, 8                   # band thresholds (verified on input)
# far chunks put on DVE (rest on ACT) per slot, for DVE/ACT balance
FAR_DVE = [7] * 4 + [7] * 4

_CACHE = {}


def _core_block(core, slot):
    return 4 * core + slot if slot < 4 else 32 + 4 * core + (slot - 4)


def _nearx_d0(slot):
    # wrapped cols with d < D2 exist only for I >= 64 - D2 + 1, i.e. slots
    # 4..7 on the highest cores; conservative shared range [4-(slot-4), D2)
    return 4 - (slot - 4)


def _chunk_table():
    """Per-slot chunk list in PSUM emission order: (kind, d).

    kind: 'N' near band d in [D0, D2); 'F' far; 'X' nearx (slots 4-7).
    Returns (table, bases, total_cols): table[slot] = list[(kind, d)],
    bases[slot] = psum column base (2 cols per chunk), A tile = slots 0-3
    cols [0, baseA_end), B tile = slots 4-7.
    """
    table = []
    for b in range(SLOTS):
        lst = [("N", d) for d in range(D0, D2)]
        lst += [("F", d) for d in range(D2, SLOT_NWIN[b])]
        if b >= 4:
            lst += [("X", d) for d in range(_nearx_d0(b), D2)]
        table.append(lst)
    basesA = []
    off = 0
    for b in range(4):
        basesA.append(off)
        off += 2 * len(table[b])
    a_end = off
    basesB = []
    off = 0
    for b in range(4, 8):
        basesB.append(off)
        off += 2 * len(table[b])
    b_end = off
    return table, basesA + basesB, a_end, b_end


def _build(ms):
    from contextlib import ExitStack
    from concourse import bacc, tile, mybir

    BF16 = mybir.dt.bfloat16
    F32 = mybir.dt.float32
    Alu = mybir.AluOpType
    Act = mybir.ActivationFunctionType

    table, bases, a_end, b_end = _chunk_table()

    nc = bacc.Bacc("TRN2", target_bir_lowering=False, debug=False,
                   num_devices=N_CORES)

    tql_ext = nc.dram_tensor("tql", [P, LC], BF16, kind="ExternalInput").ap()
    pql_ext = nc.dram_tensor("pql", [P, LC], BF16, kind="ExternalInput").ap()
    uql_ext = nc.dram_tensor("uql", [P, LC], BF16, kind="ExternalInput").ap()
    # aux f32 [P, 40]: 0:8 ti | 8:16 pi | 16:24 ui | 24:32 -ui | 32:40 ti+theta
    aux_ext = nc.dram_tensor("aux", [P, 40], F32, kind="ExternalInput").ap()
    # auxb bf16 [P,16]: col 2b = 1.0, col 2b+1 = row weights of slot b
    auxb_ext = nc.dram_tensor("auxb", [P, 16], BF16, kind="ExternalInput").ap()
    OUTC = a_end + b_end
    out_ext = nc.dram_tensor("out", [P, OUTC], BF16, kind="ExternalOutput").ap()

    # lo is a compile-time const in tq-units (ms folded into tq on host)
    lo_c = float(0.1 * 0.08 * ms)

    if True:
        with tile.TileContext(nc) as tc:
            with ExitStack() as ctx:
                singles = ctx.enter_context(tc.tile_pool(name="singles", bufs=1))
                work = ctx.enter_context(tc.tile_pool(name="work", bufs=2))
                psum = ctx.enter_context(tc.tile_pool(name="psum", bufs=1,
                                                      space="PSUM"))

                aux_sb = singles.tile([P, 40], F32)
                nc.sync.dma_start(out=aux_sb[:], in_=aux_ext[:])
                auxb_sb = singles.tile([P, 16], BF16)
                nc.sync.dma_start(out=auxb_sb[:], in_=auxb_ext[:])

                tqb = singles.tile([P, LC], BF16)
                pqb = singles.tile([P, LC], BF16)
                uqb = singles.tile([P, LC], BF16)

                # DMA issue order tuned so both engines ramp early:
                # slot-0 near data (DVE) and slot-0 ACT-far data first,
                # then the rest in consumption order.
                U0 = P * 8
                loads = [
                    (tqb, tql_ext, 256, 1024),     # slot 0 near (DVE)
                    (pqb, pql_ext, 256, 1024),
                    (uqb, uql_ext, 1024, 1664),    # slot 0 ACT far piece 1
                    (uqb, uql_ext, 1664, 2304),
                    (tqb, tql_ext, 1024, 1408),    # slots 1-3 near tail
                    (pqb, pql_ext, 1024, 1408),
                    (uqb, uql_ext, 2304, 3584),    # slot 0-3 ACT far
                    (uqb, uql_ext, 3584, 4608),    # slot 0-3 DVE far tails
                    (tqb, tql_ext, 4352, 5504),    # slots 4-7 near
                    (pqb, pql_ext, 4352, 5504),
                    (uqb, uql_ext, 4608, 5504),    # nearx + slot 4 start
                    (uqb, uql_ext, 5504, 6528),
                    (uqb, uql_ext, 6528, 7552),
                    (uqb, uql_ext, 7552, LC),
                ]
                for li, (dst, src_, c0, c1) in enumerate(loads):
                    eng = nc.sync if li % 2 == 0 else nc.gpsimd
                    eng.dma_start(out=dst[:, c0:c1], in_=src_[:, c0:c1])

                ps_a = psum.tile([P, a_end + 2], F32)
                ps_b = psum.tile([P, b_end + 2], F32)

                for b in range(SLOTS):
                    st = SLOT_START[b]
                    nwin = SLOT_NWIN[b]
                    ps = ps_a if b < 4 else ps_b
                    base = bases[b]

                    NEARC = (D2 - D0) * P       # 768 near cols
                    FULLC = (D1 - D0) * P       # 384 masked cols
                    n0 = st + D0 * P

                    m_t = work.tile([P, NEARC], BF16, tag="m")
                    pdn = work.tile([P, NEARC], BF16, tag="pdn")
                    vp = work.tile([P, NEARC], BF16, tag="vp")
                    viol = work.tile([P, NEARC], BF16, tag="viol")
                    v_t = work.tile([P, FULLC], BF16, tag="v")
                    g_t = work.tile([P, FULLC], BF16, tag="g")

                    # sorted order: tq_j >= ti on non-wrapped near cols
                    nc.vector.tensor_scalar(
                        out=m_t[:], in0=tqb[:, n0:n0 + NEARC],
                        scalar1=aux_sb[:, b:b + 1], scalar2=lo_c,
                        op0=Alu.subtract, op1=Alu.max)
                    nc.vector.tensor_scalar(
                        out=pdn[:], in0=pqb[:, n0:n0 + NEARC],
                        scalar1=aux_sb[:, 8 + b:9 + b], scalar2=None,
                        op0=Alu.subtract)
                    nc.vector.tensor_tensor(
                        out=vp[:], in0=m_t[:], in1=pdn[:], op=Alu.subtract)
                    nc.vector.tensor_scalar(
                        out=viol[:], in0=vp[:], scalar1=0.0, scalar2=None,
                        op0=Alu.max)
                    nc.vector.tensor_scalar(
                        out=v_t[:], in0=tqb[:, n0:n0 + FULLC],
                        scalar1=aux_sb[:, 32 + b:33 + b], scalar2=None,
                        op0=Alu.is_ge)
                    nc.vector.tensor_tensor(
                        out=g_t[:], in0=viol[:, 0:FULLC], in1=v_t[:],
                        op=Alu.mult)

                    # far: h = |uq_j - u_i|, split ACT (bulk) / DVE (tail)
                    nfar = nwin - D2
                    FARC = nfar * P
                    f0 = st + D2 * P
                    h_t = work.tile([P, FARC], BF16, tag="h")
                    nd = FAR_DVE[b]
                    splitc = (nfar - nd) * P
                    # ACT: relu(u_i - uq_j) (true viol on non-wrapped cols),
                    # two pieces so the first starts as soon as DMA lands
                    sp1 = (splitc // (2 * P)) * P
                    if b == 0:
                        sp1 = 5 * P   # small first piece: earliest ACT start
                    nc.scalar.activation(
                        out=h_t[:, 0:sp1], in_=uqb[:, f0:f0 + sp1],
                        func=Act.Relu,
                        bias=aux_sb[:, 16 + b:17 + b], scale=-1.0)
                    nc.scalar.activation(
                        out=h_t[:, sp1:splitc], in_=uqb[:, f0 + sp1:f0 + splitc],
                        func=Act.Relu,
                        bias=aux_sb[:, 16 + b:17 + b], scale=-1.0)
                    # DVE: relu(uq_j - u_i) (true viol on wrapped cols)
                    nc.vector.tensor_scalar(
                        out=h_t[:, splitc:FARC],
                        in0=uqb[:, f0 + splitc:f0 + FARC],
                        scalar1=aux_sb[:, 16 + b:17 + b], scalar2=0.0,
                        op0=Alu.subtract, op1=Alu.max)

                    hx_t = None
                    if b >= 4:
                        xd0 = _nearx_d0(b)
                        XC = (D2 - xd0) * P
                        x0 = st + xd0 * P
                        hx_t = work.tile([P, XC], BF16, tag="hx")
                        nc.vector.tensor_scalar(
                            out=hx_t[:], in0=uqb[:, x0:x0 + XC],
                            scalar1=aux_sb[:, 16 + b:17 + b], scalar2=0.0,
                            op0=Alu.subtract, op1=Alu.max)

                    # PE reduce per chunk, in _chunk_table order
                    for c, (kind, d) in enumerate(table[b]):
                        if kind == "N":
                            ci = d - D0
                            src = (g_t[:, ci * P:(ci + 1) * P] if d < D1 else
                                   viol[:, ci * P:(ci + 1) * P])
                        elif kind == "F":
                            ci = d - D2
                            src = h_t[:, ci * P:(ci + 1) * P]
                        else:
                            ci = d - _nearx_d0(b)
                            src = hx_t[:, ci * P:(ci + 1) * P]
                        nc.tensor.matmul(
                            ps[:, base + 2 * c:base + 2 * c + 2],
                            lhsT=src, rhs=auxb_sb[:, 2 * b:2 * b + 2],
                            start=True, stop=True)

                # drain pushers into the 2 spare cols of each psum tile
                pushers = []
                for ps, spare, b in ((ps_a, a_end, 3), (ps_b, b_end, 7)):
                    pushers.append(nc.tensor.matmul(
                        ps[:, spare:spare + 2],
                        lhsT=uqb[:, U0:U0 + P],
                        rhs=auxb_sb[:, 2 * b:2 * b + 2],
                        start=True, stop=True))

                out_sb = singles.tile([P, OUTC], BF16)
                c0 = nc.scalar.copy(out=out_sb[:, 0:a_end], in_=ps_a[:, 0:a_end])
                c1 = nc.scalar.copy(out=out_sb[:, a_end:OUTC],
                                    in_=ps_b[:, 0:b_end])
                for cc in (c0, c1):
                    for pp in pushers:
                        tile.add_dep_helper(cc.ins, pp.ins,
                                            reason="final copy waits pushers")
                nc.sync.dma_start(out=out_ext[:], in_=out_sb[:])

    nc.compile()
    return nc


def _get_nc(ms=1.0):
    key = ("nc", float(ms))
    if key not in _CACHE:
        _CACHE[key] = _build(float(ms))
    return _CACHE[key]


def _sorted_quantized(predictions, targets, snr_weights, margin_scale):
    ms = float(margin_scale)
    bf16 = ml_dtypes.bfloat16
    t = np.asarray(targets, np.float32)
    p = np.asarray(predictions, np.float32)
    w = np.asarray(snr_weights, np.float32)
    order = np.argsort(t, kind="stable")
    tso, pso, wso = t[order], p[order], w[order]
    tq = (0.08 * ms * tso).astype(bf16)
    tqf = tq.astype(np.float32)
    uq = (pso - tqf).astype(bf16)
    pq = pso.astype(bf16)
    wq = wso.astype(bf16)
    return tso, tq, uq, pq, wq, ms


def _check_bands(tso):
    """Verify the compiled band predicates on the actual sorted targets."""
    ts = tso.astype(np.float64)
    NB = NBLOCKS_TOTAL
    for I in range(NB):
        nwin = 33 if I <= 31 else 32
        for d in range(nwin):
            J = I + d
            if J >= NB:
                # wrapped cols must be far-class; nearx coverage of wrapped
                # d < D2 is structural (d >= 64 - I >= _nearx_d0(slot))
                if ts[I * P] - ts[(J - NB) * P + P - 1] < 0.1:
                    return False
                continue
            if d < D0:
                if ts[J * P + P - 1] - ts[I * P] >= 0.05:
                    return False
            elif D1 <= d < D2:
                if ts[J * P] - ts[I * P + P - 1] < 0.05:
                    return False
            elif d >= D2:
                if ts[J * P] - ts[I * P + P - 1] < 0.1:
                    return False
    return True


def _prepare_in_maps(predictions, targets, snr_weights, margin_scale):
    bf16 = ml_dtypes.bfloat16
    tso, tq, uq, pq, wq, ms = _sorted_quantized(
        predictions, targets, snr_weights, margin_scale)
    tqf = tq.astype(np.float32)
    uqf = uq.astype(np.float32)
    pqf = pq.astype(np.float32)
    theta = np.float32(0.05 * 0.08 * ms)

    in_maps = []
    for core in range(N_CORES):
        rot = 4 * core * P
        idx = (rot + np.arange(LC)) % N
        tql = np.ascontiguousarray(
            np.broadcast_to(tq[idx].reshape(1, LC), (P, LC)))
        pql = np.ascontiguousarray(
            np.broadcast_to(pq[idx].reshape(1, LC), (P, LC)))
        uql = np.ascontiguousarray(
            np.broadcast_to(uq[idx].reshape(1, LC), (P, LC)))
        ti = np.empty((P, SLOTS), np.float32)
        pi = np.empty((P, SLOTS), np.float32)
        ui = np.empty((P, SLOTS), np.float32)
        wib = np.empty((P, SLOTS), np.float32)
        for slot in range(SLOTS):
            I = _core_block(core, slot)
            rows = slice(I * P, (I + 1) * P)
            ti[:, slot] = tqf[rows]
            pi[:, slot] = pqf[rows]
            ui[:, slot] = uqf[rows]
            wib[:, slot] = wq[rows].astype(np.float32)
        aux = np.concatenate([ti, pi, ui, -ui, ti + theta], axis=1)
        auxw = np.empty((P, 2 * SLOTS), np.float32)
        auxw[:, 0::2] = 1.0
        auxw[:, 1::2] = wib
        in_maps.append({"tql": tql, "pql": pql, "uql": uql,
                        "aux": aux.astype(np.float32),
                        "auxb": auxw.astype(bf16)})
    return in_maps


def _numpy_fallback(predictions, targets, snr_weights, margin_scale):
    t = np.asarray(targets, np.float64)
    p = np.asarray(predictions, np.float64)
    w = np.asarray(snr_weights, np.float64)
    ms = float(margin_scale)
    total = 0.0
    count = 0
    for i0 in range(0, N, 512):
        i1 = min(i0 + 512, N)
        td = t[i0:i1, None] - t[None, :]
        ad = np.abs(td)
        upper = (np.arange(i0, i1)[:, None] < np.arange(N)[None, :])
        valid = upper & (ad >= 0.05)
        margin = ms * 0.08 * np.clip(ad, 0.1, 1.0)
        pdm = p[i0:i1, None] - p[None, :]
        viol = np.maximum(-np.sign(td) * pdm + margin, 0.0)
        pw = 0.5 * (w[i0:i1, None] + w[None, :])
        total += float((pw * viol)[valid].sum())
        count += int(valid.sum())
    return np.float32(total / count if count > 0 else 0.0)


def kernel(predictions, targets, snr_weights, margin_scale):
    from concourse.bass_utils import run_bass_kernel_spmd

    if float(margin_scale) <= 0.0:
        return _numpy_fallback(predictions, targets, snr_weights, margin_scale)

    tso, tq, uq, pq, wq, ms = _sorted_quantized(
        predictions, targets, snr_weights, margin_scale)
    if not _check_bands(tso):
        return _numpy_fallback(predictions, targets, snr_weights, margin_scale)

    nc = _get_nc(ms)
    in_maps = _prepare_in_maps(predictions, targets, snr_weights, margin_scale)
    res = run_bass_kernel_spmd(nc, in_maps, core_ids=list(range(N_CORES)))

    uqd = uq.astype(np.float64)
    wqd = wq.astype(np.float64)
    Su = uqd.reshape(NBLOCKS_TOTAL, P).sum(axis=1)
    Sw = wqd.reshape(NBLOCKS_TOTAL, P).sum(axis=1)
    Swu = (wqd * uqd).reshape(NBLOCKS_TOTAL, P).sum(axis=1)

    table, bases, a_end, b_end = _chunk_table()

    # exact pair count via sorted two-pointer over raw targets (f64)
    ts_sorted = np.sort(np.asarray(targets, np.float64))
    C = float(np.searchsorted(ts_sorted, ts_sorted - 0.05, side="right").sum())

    total = 0.0
    for core in range(N_CORES):
        o = np.asarray(res.results[core]["out"], np.float64)
        for b in range(SLOTS):
            I = _core_block(core, b)
            off = bases[b] + (0 if b < 4 else a_end)
            for c, (kind, d) in enumerate(table[b]):
                wrapped = (I + d) >= NBLOCKS_TOTAL
                if kind == "N" and wrapped:
                    continue
                if kind == "X" and not wrapped:
                    continue
                J = (I + d) % NBLOCKS_TOTAL
                colsum = o[:, off + 2 * c]
                wcolsum = o[:, off + 2 * c + 1]
                wcol = wqd[J * P:(J + 1) * P]
                A = 0.5 * (wcol @ colsum + wcolsum.sum())
                if kind == "N":
                    total += A
                else:
                    # device computed relu(sigma_dev*x): ACT chunks +x,
                    # DVE/nearx chunks -x.  Correct mismatched orientation
                    # with the exact bilinear B = sum w_bar * x.
                    if kind == "F":
                        nfar = SLOT_NWIN[b] - D2
                        on_act = (d - D2) < (nfar - FAR_DVE[b])
                    else:
                        on_act = False
                    B = 0.5 * (P * Swu[I] + Sw[J] * Su[I]
                               - Sw[I] * Su[J] - P * Swu[J])
                    if on_act:
                        corr = -B if wrapped else 0.0
                    else:
                        corr = B if not wrapped else 0.0
                    total += A + corr

    loss = total / C if C > 0 else 0.0
    return np.float32(loss)


# revision 21
# speedup vs baseline: 3.2415x; 1.0927x over previous
"""AdaptiveRankingLoss on 8 Trainium2 NeuronCores (Bass/Tile), v7 "sorted-u".

Math
----
reference:  loss = sum_{i<j, |t_i-t_j|>=0.05} 0.5*(w_i+w_j)*relu(-sign(td)*pd + m) / count
            td = t_i - t_j, pd = p_i - p_j, m = ms*0.08*clip(|td|, 0.1, 1.0)

v7 key idea: sort everything by target on the host.  In sorted order, for a
pair (i, j) with sorted ranks r_i < r_j, the violation is
    viol = relu(p_i - p_j + m),   m = max(ad, lo),  ad = tq_j - tq_i >= 0
with tq = 0.08*ms*t.  Define u = p - tq.  Whenever the pair is guaranteed
valid (ad >= theta) and unclipped (ad >= lo), m == ad and
    viol = relu(u_i - u_j)          -- ONE fused element op.
Using relu(s*x) = (s*x + |x|)/2, the device only computes h = |u_i - u_j|
(orientation-free!) and the host adds the exact bilinear term
sum 0.5*(w_i+w_j)*s*(u_i-u_j) from per-block prefix sums in f64.

The 64x64 grid of 128-row blocks uses the v6 circulant schedule: row-block I
processes column-blocks J = (I+d) % 64 for d in [0, n_I), n_I = 33 (I<=31)
else 32; each unordered block pair lands in exactly one window.  Core k owns
blocks {4k..4k+3, 32+4k..32+4k+3}; column data is laid out per-core rotated
by 4k blocks with a 3-block duplicated tail, so every window is contiguous.

Per-window distance bands (host-verified predicates, shared program):
    d in [0, D0):  skip    -- every pair invalid (|dt| < 0.05)
    d in [D0,D1):  full    -- masked chain (some pairs invalid)
    d in [D1,D2):  mid     -- all valid, margin may clip (no mask)
    d in [D2, n):  far     -- all valid, unclipped: h = |u_i - u_j| only
    wrapped cols (I+d >= 64) at d < D2: covered by an extra "nearx" abs
    range on slots 4-7; the near-chain mask v = (tq_j >= t_i + theta) is
    one-sided so wrapped cols contribute exactly 0 there.  Chunks whose
    band treatment does not match their true class are simply ignored by
    the host (each block pair is USED from exactly one chunk).

Near chain (bf16, rows on partitions):
    DVE ts : m    = abs_max(tq_j - ti, lo)        [d in [D0,D2)]
    DVE ts : pdn  = pq_j - pi
    DVE tt : vp   = m - pdn
    DVE ts : viol = max(vp, 0)
    DVE ts : v    = (tq_j is_ge ti+theta)         [d in [D0,D1) only]
    DVE tt : g    = viol * v                      [d in [D0,D1) only]
Far/nearx: h = |uq_j - u_i| as DVE ts-dual (subtract, abs_max 0) on a
~17% column split and ACT Abs(bias=-u_i) on the rest (engine balance).

PE reduces every 128-col chunk with rhs=[ones, w_row]: ps[:,2c:2c+2] =
[colsum, w_i-weighted colsum].  Host combines in f64:
    near chunk:  0.5*(w_col @ colsum + sum(wcolsum))
    far  chunk:  0.5*( sigma*B(I,J) + 0.5*(w_col @ colsum + sum(wcolsum)) )
with B(I,J) = 0.5*(128*Swu_I + Sw_J*Su_I - Sw_I*Su_J - 128*Swu_J) from
per-block sums of the SAME bf16-quantized u and w, sigma = -1 iff wrapped.
Count C is exact on host (sorted two-pointer over raw targets).
"""

import sys

if "/opt/trn_rl_repo" not in sys.path:
    sys.path.insert(0, "/opt/trn_rl_repo")

import numpy as np
import ml_dtypes

N = 8192
P = 128
N_CORES = 8
NBLOCKS_TOTAL = N // P                 # 64 row blocks globally
SLOTS = 8                              # row blocks per core
LC = N + 3 * P                         # 8576 local (rotated) columns
SLOT_START = [P * i for i in range(4)] + [N // 2 + P * i for i in range(4)]
SLOT_NWIN = [33] * 4 + [32] * 4
D0, D1, D2 = 2, 5, 8                   # band thresholds (verified on input)
# far chunks put on DVE (rest on ACT) per slot, for DVE/ACT balance
FAR_DVE = [7] * 4 + [7] * 4

_CACHE = {}


def _core_block(core, slot):
    return 4 * core + slot if slot < 4 else 32 + 4 * core + (slot - 4)


def _nearx_d0(slot):
    # wrapped cols with d < D2 exist only for I >= 64 - D2 + 1, i.e. slots
    # 4..7 on the highest cores; conservative shared range [4-(slot-4), D2)
    return 4 - (slot - 4)


def _chunk_table():
    """Per-slot chunk list in PSUM emission order: (kind, d).

    kind: 'N' near band d in [D0, D2); 'F' far; 'X' nearx (slots 4-7).
    Returns (table, bases, total_cols): table[slot] = list[(kind, d)],
    bases[slot] = psum column base (2 cols per chunk), A tile = slots 0-3
    cols [0, baseA_end), B tile = slots 4-7.
    """
    table = []
    for b in range(SLOTS):
        lst = [("N", d) for d in range(D0, D2)]
        lst += [("F", d) for d in range(D2, SLOT_NWIN[b])]
        if b >= 4:
            lst += [("X", d) for d in range(_nearx_d0(b), D2)]
        table.append(lst)
    basesA = []
    off = 0
    for b in range(4):
        basesA.append(off)
        off += 2 * len(table[b])
    a_end = off
    basesB = []
    off = 0
    for b in range(4, 8):
        basesB.append(off)
        off += 2 * len(table[b])
    b_end = off
    return table, basesA + basesB, a_end, b_end


def _build(ms):
    from contextlib import ExitStack
    from concourse import bacc, tile, mybir

    BF16 = mybir.dt.bfloat16
    F32 = mybir.dt.float32
    Alu = mybir.AluOpType
    Act = mybir.ActivationFunctionType

    table, bases, a_end, b_end = _chunk_table()

    nc = bacc.Bacc("TRN2", target_bir_lowering=False, debug=False,
                   num_devices=N_CORES)

    tql_ext = nc.dram_tensor("tql", [P, LC], BF16, kind="ExternalInput").ap()
    pql_ext = nc.dram_tensor("pql", [P, LC], BF16, kind="ExternalInput").ap()
    uql_ext = nc.dram_tensor("uql", [P, LC], BF16, kind="ExternalInput").ap()
    # aux f32 [P, 40]: 0:8 ti | 8:16 pi | 16:24 ui | 24:32 -ui | 32:40 ti+theta
    aux_ext = nc.dram_tensor("aux", [P, 40], F32, kind="ExternalInput").ap()
    # auxb bf16 [P,16]: col 2b = 1.0, col 2b+1 = row weights of slot b
    auxb_ext = nc.dram_tensor("auxb", [P, 16], BF16, kind="ExternalInput").ap()
    OUTC = a_end + b_end
    out_ext = nc.dram_tensor("out", [P, OUTC], BF16, kind="ExternalOutput").ap()

    # lo is a compile-time const in tq-units (ms folded into tq on host)
    lo_c = float(0.1 * 0.08 * ms)

    if True:
        with tile.TileContext(nc) as tc:
            with ExitStack() as ctx:
                singles = ctx.enter_context(tc.tile_pool(name="singles", bufs=1))
                work = ctx.enter_context(tc.tile_pool(name="work", bufs=2))
                psum = ctx.enter_context(tc.tile_pool(name="psum", bufs=1,
                                                      space="PSUM"))

                # aux triggers ride the scalar queue (ACT waits on uq anyway)
                # so the sync/gpsimd queues stream the big loads immediately
                aux_sb = singles.tile([P, 40], F32)
                nc.scalar.dma_start(out=aux_sb[:], in_=aux_ext[:])
                auxb_sb = singles.tile([P, 16], BF16)
                nc.scalar.dma_start(out=auxb_sb[:], in_=auxb_ext[:])

                tqb = singles.tile([P, LC], BF16)
                pqb = singles.tile([P, LC], BF16)
                uqb = singles.tile([P, LC], BF16)

                # DMA issue order tuned so both engines ramp early:
                # slot-0 near data (DVE) and slot-0 ACT-far data first,
                # then the rest in consumption order.
                U0 = P * 8
                loads = [
                    (tqb, tql_ext, 256, 1024),     # slot 0 near (DVE)
                    (pqb, pql_ext, 256, 1024),
                    (uqb, uql_ext, 1024, 1664),    # slot 0 ACT far piece 1
                    (uqb, uql_ext, 1664, 2304),
                    (tqb, tql_ext, 1024, 1408),    # slots 1-3 near tail
                    (pqb, pql_ext, 1024, 1408),
                    (uqb, uql_ext, 2304, 3584),    # slot 0-3 ACT far
                    (uqb, uql_ext, 3584, 4608),    # slot 0-3 DVE far tails
                    (tqb, tql_ext, 4352, 5504),    # slots 4-7 near
                    (pqb, pql_ext, 4352, 5504),
                    (uqb, uql_ext, 4608, 5504),    # nearx + slot 4 start
                    (uqb, uql_ext, 5504, 6528),
                    (uqb, uql_ext, 6528, 7552),
                    (uqb, uql_ext, 7552, LC),
                ]
                for li, (dst, src_, c0, c1) in enumerate(loads):
                    eng = nc.sync if li % 2 == 0 else nc.gpsimd
                    eng.dma_start(out=dst[:, c0:c1], in_=src_[:, c0:c1])

                ps_a = psum.tile([P, a_end + 2], F32)
                ps_b = psum.tile([P, b_end + 2], F32)

                for b in range(SLOTS):
                    st = SLOT_START[b]
                    nwin = SLOT_NWIN[b]
                    ps = ps_a if b < 4 else ps_b
                    base = bases[b]

                    NEARC = (D2 - D0) * P       # 768 near cols
                    FULLC = (D1 - D0) * P       # 384 masked cols
                    n0 = st + D0 * P

                    m_t = work.tile([P, NEARC], BF16, tag="m")
                    pdn = work.tile([P, NEARC], BF16, tag="pdn")
                    vp = work.tile([P, NEARC], BF16, tag="vp")
                    viol = work.tile([P, NEARC], BF16, tag="viol")
                    v_t = work.tile([P, FULLC], BF16, tag="v")
                    g_t = work.tile([P, FULLC], BF16, tag="g")

                    # sorted order: tq_j >= ti on non-wrapped near cols
                    nc.vector.tensor_scalar(
                        out=m_t[:], in0=tqb[:, n0:n0 + NEARC],
                        scalar1=aux_sb[:, b:b + 1], scalar2=lo_c,
                        op0=Alu.subtract, op1=Alu.max)
                    nc.vector.tensor_scalar(
                        out=pdn[:], in0=pqb[:, n0:n0 + NEARC],
                        scalar1=aux_sb[:, 8 + b:9 + b], scalar2=None,
                        op0=Alu.subtract)
                    nc.vector.tensor_tensor(
                        out=vp[:], in0=m_t[:], in1=pdn[:], op=Alu.subtract)
                    nc.vector.tensor_scalar(
                        out=viol[:], in0=vp[:], scalar1=0.0, scalar2=None,
                        op0=Alu.max)
                    nc.vector.tensor_scalar(
                        out=v_t[:], in0=tqb[:, n0:n0 + FULLC],
                        scalar1=aux_sb[:, 32 + b:33 + b], scalar2=None,
                        op0=Alu.is_ge)
                    nc.vector.tensor_tensor(
                        out=g_t[:], in0=viol[:, 0:FULLC], in1=v_t[:],
                        op=Alu.mult)

                    # far: h = |uq_j - u_i|, split ACT (bulk) / DVE (tail)
                    nfar = nwin - D2
                    FARC = nfar * P
                    f0 = st + D2 * P
                    h_t = work.tile([P, FARC], BF16, tag="h")
                    nd = FAR_DVE[b]
                    splitc = (nfar - nd) * P
                    # ACT: relu(u_i - uq_j) (true viol on non-wrapped cols),
                    # two pieces so the first starts as soon as DMA lands
                    sp1 = (splitc // (2 * P)) * P
                    if b == 0:
                        sp1 = 5 * P   # small first piece: earliest ACT start
                    nc.scalar.activation(
                        out=h_t[:, 0:sp1], in_=uqb[:, f0:f0 + sp1],
                        func=Act.Relu,
                        bias=aux_sb[:, 16 + b:17 + b], scale=-1.0)
                    nc.scalar.activation(
                        out=h_t[:, sp1:splitc], in_=uqb[:, f0 + sp1:f0 + splitc],
                        func=Act.Relu,
                        bias=aux_sb[:, 16 + b:17 + b], scale=-1.0)
                    # DVE: relu(uq_j - u_i) (true viol on wrapped cols)
                    nc.vector.tensor_scalar(
                        out=h_t[:, splitc:FARC],
                        in0=uqb[:, f0 + splitc:f0 + FARC],
                        scalar1=aux_sb[:, 16 + b:17 + b], scalar2=0.0,
                        op0=Alu.subtract, op1=Alu.max)

                    hx_t = None
                    if b >= 4:
                        xd0 = _nearx_d0(b)
                        XC = (D2 - xd0) * P
                        x0 = st + xd0 * P
                        hx_t = work.tile([P, XC], BF16, tag="hx")
                        nc.vector.tensor_scalar(
                            out=hx_t[:], in0=uqb[:, x0:x0 + XC],
                            scalar1=aux_sb[:, 16 + b:17 + b], scalar2=0.0,
                            op0=Alu.subtract, op1=Alu.max)

                    # PE reduce per chunk, in _chunk_table order
                    for c, (kind, d) in enumerate(table[b]):
                        if kind == "N":
                            ci = d - D0
                            src = (g_t[:, ci * P:(ci + 1) * P] if d < D1 else
                                   viol[:, ci * P:(ci + 1) * P])
                        elif kind == "F":
                            ci = d - D2
                            src = h_t[:, ci * P:(ci + 1) * P]
                        else:
                            ci = d - _nearx_d0(b)
                            src = hx_t[:, ci * P:(ci + 1) * P]
                        nc.tensor.matmul(
                            ps[:, base + 2 * c:base + 2 * c + 2],
                            lhsT=src, rhs=auxb_sb[:, 2 * b:2 * b + 2],
                            start=True, stop=True)

                # drain pushers into the 2 spare cols of each psum tile
                pushers = []
                for ps, spare, b in ((ps_a, a_end, 3), (ps_b, b_end, 7)):
                    pushers.append(nc.tensor.matmul(
                        ps[:, spare:spare + 2],
                        lhsT=uqb[:, U0:U0 + P],
                        rhs=auxb_sb[:, 2 * b:2 * b + 2],
                        start=True, stop=True))

                out_sb = singles.tile([P, OUTC], BF16)
                c0 = nc.vector.tensor_copy(out=out_sb[:, 0:a_end],
                                           in_=ps_a[:, 0:a_end])
                c1 = nc.vector.tensor_copy(out=out_sb[:, a_end:OUTC],
                                           in_=ps_b[:, 0:b_end])
                for cc in (c0, c1):
                    for pp in pushers:
                        tile.add_dep_helper(cc.ins, pp.ins,
                                            reason="final copy waits pushers")
                nc.sync.dma_start(out=out_ext[:], in_=out_sb[:])

    nc.compile()
    return nc


def _get_nc(ms=1.0):
    key = ("nc", float(ms))
    if key not in _CACHE:
        _CACHE[key] = _build(float(ms))
    return _CACHE[key]


def _sorted_quantized(predictions, targets, snr_weights, margin_scale):
    ms = float(margin_scale)
    bf16 = ml_dtypes.bfloat16
    t = np.asarray(targets, np.float32)
    p = np.asarray(predictions, np.float32)
    w = np.asarray(snr_weights, np.float32)
    order = np.argsort(t, kind="stable")
    tso, pso, wso = t[order], p[order], w[order]
    tq = (0.08 * ms * tso).astype(bf16)
    tqf = tq.astype(np.float32)
    uq = (pso - tqf).astype(bf16)
    pq = pso.astype(bf16)
    wq = wso.astype(bf16)
    return tso, tq, uq, pq, wq, ms


def _check_bands(tso):
    """Verify the compiled band predicates on the actual sorted targets."""
    ts = tso.astype(np.float64)
    NB = NBLOCKS_TOTAL
    for I in range(NB):
        nwin = 33 if I <= 31 else 32
        for d in range(nwin):
            J = I + d
            if J >= NB:
                # wrapped cols must be far-class; nearx coverage of wrapped
                # d < D2 is structural (d >= 64 - I >= _nearx_d0(slot))
                if ts[I * P] - ts[(J - NB) * P + P - 1] < 0.1:
                    return False
                continue
            if d < D0:
                if ts[J * P + P - 1] - ts[I * P] >= 0.05:
                    return False
            elif D1 <= d < D2:
                if ts[J * P] - ts[I * P + P - 1] < 0.05:
                    return False
            elif d >= D2:
                if ts[J * P] - ts[I * P + P - 1] < 0.1:
                    return False
    return True


def _prepare_in_maps(predictions, targets, snr_weights, margin_scale):
    bf16 = ml_dtypes.bfloat16
    tso, tq, uq, pq, wq, ms = _sorted_quantized(
        predictions, targets, snr_weights, margin_scale)
    tqf = tq.astype(np.float32)
    uqf = uq.astype(np.float32)
    pqf = pq.astype(np.float32)
    theta = np.float32(0.05 * 0.08 * ms)

    in_maps = []
    for core in range(N_CORES):
        rot = 4 * core * P
        idx = (rot + np.arange(LC)) % N
        tql = np.ascontiguousarray(
            np.broadcast_to(tq[idx].reshape(1, LC), (P, LC)))
        pql = np.ascontiguousarray(
            np.broadcast_to(pq[idx].reshape(1, LC), (P, LC)))
        uql = np.ascontiguousarray(
            np.broadcast_to(uq[idx].reshape(1, LC), (P, LC)))
        ti = np.empty((P, SLOTS), np.float32)
        pi = np.empty((P, SLOTS), np.float32)
        ui = np.empty((P, SLOTS), np.float32)
        wib = np.empty((P, SLOTS), np.float32)
        for slot in range(SLOTS):
            I = _core_block(core, slot)
            rows = slice(I * P, (I + 1) * P)
            ti[:, slot] = tqf[rows]
            pi[:, slot] = pqf[rows]
            ui[:, slot] = uqf[rows]
            wib[:, slot] = wq[rows].astype(np.float32)
        aux = np.concatenate([ti, pi, ui, -ui, ti + theta], axis=1)
        auxw = np.empty((P, 2 * SLOTS), np.float32)
        auxw[:, 0::2] = 1.0
        auxw[:, 1::2] = wib
        in_maps.append({"tql": tql, "pql": pql, "uql": uql,
                        "aux": aux.astype(np.float32),
                        "auxb": auxw.astype(bf16)})
    return in_maps


def _numpy_fallback(predictions, targets, snr_weights, margin_scale):
    t = np.asarray(targets, np.float64)
    p = np.asarray(predictions, np.float64)
    w = np.asarray(snr_weights, np.float64)
    ms = float(margin_scale)
    total = 0.0
    count = 0
    for i0 in range(0, N, 512):
        i1 = min(i0 + 512, N)
        td = t[i0:i1, None] - t[None, :]
        ad = np.abs(td)
        upper = (np.arange(i0, i1)[:, None] < np.arange(N)[None, :])
        valid = upper & (ad >= 0.05)
        margin = ms * 0.08 * np.clip(ad, 0.1, 1.0)
        pdm = p[i0:i1, None] - p[None, :]
        viol = np.maximum(-np.sign(td) * pdm + margin, 0.0)
        pw = 0.5 * (w[i0:i1, None] + w[None, :])
        total += float((pw * viol)[valid].sum())
        count += int(valid.sum())
    return np.float32(total / count if count > 0 else 0.0)


def kernel(predictions, targets, snr_weights, margin_scale):
    from concourse.bass_utils import run_bass_kernel_spmd

    if float(margin_scale) <= 0.0:
        return _numpy_fallback(predictions, targets, snr_weights, margin_scale)

    tso, tq, uq, pq, wq, ms = _sorted_quantized(
        predictions, targets, snr_weights, margin_scale)
    if not _check_bands(tso):
        return _numpy_fallback(predictions, targets, snr_weights, margin_scale)

    nc = _get_nc(ms)
    in_maps = _prepare_in_maps(predictions, targets, snr_weights, margin_scale)
    res = run_bass_kernel_spmd(nc, in_maps, core_ids=list(range(N_CORES)))

    uqd = uq.astype(np.float64)
    wqd = wq.astype(np.float64)
    Su = uqd.reshape(NBLOCKS_TOTAL, P).sum(axis=1)
    Sw = wqd.reshape(NBLOCKS_TOTAL, P).sum(axis=1)
    Swu = (wqd * uqd).reshape(NBLOCKS_TOTAL, P).sum(axis=1)

    table, bases, a_end, b_end = _chunk_table()

    # exact pair count via sorted two-pointer over raw targets (f64)
    ts_sorted = np.sort(np.asarray(targets, np.float64))
    C = float(np.searchsorted(ts_sorted, ts_sorted - 0.05, side="right").sum())

    total = 0.0
    for core in range(N_CORES):
        o = np.asarray(res.results[core]["out"], np.float64)
        for b in range(SLOTS):
            I = _core_block(core, b)
            off = bases[b] + (0 if b < 4 else a_end)
            for c, (kind, d) in enumerate(table[b]):
                wrapped = (I + d) >= NBLOCKS_TOTAL
                if kind == "N" and wrapped:
                    continue
                if kind == "X" and not wrapped:
                    continue
                J = (I + d) % NBLOCKS_TOTAL
                colsum = o[:, off + 2 * c]
                wcolsum = o[:, off + 2 * c + 1]
                wcol = wqd[J * P:(J + 1) * P]
                A = 0.5 * (wcol @ colsum + wcolsum.sum())
                if kind == "N":
                    total += A
                else:
                    # device computed relu(sigma_dev*x): ACT chunks +x,
                    # DVE/nearx chunks -x.  Correct mismatched orientation
                    # with the exact bilinear B = sum w_bar * x.
                    if kind == "F":
                        nfar = SLOT_NWIN[b] - D2
                        on_act = (d - D2) < (nfar - FAR_DVE[b])
                    else:
                        on_act = False
                    B = 0.5 * (P * Swu[I] + Sw[J] * Su[I]
                               - Sw[I] * Su[J] - P * Swu[J])
                    if on_act:
                        corr = -B if wrapped else 0.0
                    else:
                        corr = B if not wrapped else 0.0
                    total += A + corr

    loss = total / C if C > 0 else 0.0
    return np.float32(loss)
